# revision 16
# baseline (speedup 1.0000x reference)
"""Trainium2 Bass kernel for nn_KSpaceLoss: exact type-2 NUFFT k-space loss.

loss = 0.1 * (sum|d| / sum|a|) + 0.1 * sqrt(sum d^2 / sum a^2)
  d = (E @ x) * mask - kdata * mask,  a = kdata * mask
  E[k, n] = exp(-2j*pi * traj[:, k] . r[:, n])   (K=8192, N=96*96)

Sharding: K axis split across 8 NeuronCores (1024 samples each).

v2 structure (per core):
 - Mirror pairing: E(-r) = conj(E(r)); host pairs grid points r/-r, so only
   4704 representative points (38 chunks of 128, padded) need phase/trig.
   Paired contribution with u± = xr ± xr', v± = xi ± xi':
     Re += er*u+ + ei*(-v-) ;  Im += er*v+ + ei*u-
 - PE: ph = 4-row split-precision phase matmul (fp32 PSUM)
       rnd = 5-row matmul with MAGIC row last -> fl(phase+MAGIC) exactly
 - VE: mf = (rnd - MAGIC) - ph = -frac(phase)  (f16), uu = |mf| (bitand)
 - ACT: er = sin(pi/2 - 2pi*uu) = cos(2pi*ph); ei = sin(2pi*mf) = -sin(2pi*ph)
 - PE: ps[0:64, k] accumulates er-stream @ [u+;v+] and ei-stream @ [-v-;u-]
 - residual: d = ps*mask - kdata*mask on (64,k); L2 partials via accum;
   |d| via fp32 pairing matmul (dr^2+di^2) + sqrt-accum. a-side sums on host.
"""

import math

import numpy as np
import ml_dtypes

import concourse.bacc as bacc
import concourse.tile as tile
from concourse import mybir
from concourse.bass_utils import run_bass_kernel_spmd

X, Y, Z = 96, 96, 1
C, S, T = 8, 1, 4
K = 8192
N = X * Y * Z
NCORES = 8
KL = K // NCORES          # 1024 k-samples per core
NR = 4864                 # padded representative points (38 chunks)
NCH = NR // 128           # 38
SC = 2                    # chunks per sin supertile
CST = C * S * T           # 32
W1, W2 = 0.1, 0.1

F32 = mybir.dt.float32
F8 = mybir.dt.float8e4
U16 = mybir.dt.uint16
F16 = mybir.dt.float16
BF16 = mybir.dt.bfloat16
PI = math.pi
MAGIC = 12582912.0          # 1.5 * 2^23: fl(x + MAGIC) - MAGIC == round(x)


def build_kernel():
    nc = bacc.Bacc("TRN2", target_bir_lowering=False, debug=False,
                   num_devices=NCORES)

    w1_d = nc.dram_tensor("w1", [128, NCH, 64], F8, kind="ExternalInput").ap()
    w2_d = nc.dram_tensor("w2", [128, NCH, 64], F8, kind="ExternalInput").ap()
    r2_d = nc.dram_tensor("r2", [4, NR], BF16, kind="ExternalInput").ap()
    tw_d = nc.dram_tensor("tw", [4, KL], BF16, kind="ExternalInput").ap()
    pso_d = nc.dram_tensor("pso", [64, KL], F32, kind="ExternalOutput").ap()

    Sin = mybir.ActivationFunctionType.Sin
    Sqrt = mybir.ActivationFunctionType.Sqrt
    Ident = mybir.ActivationFunctionType.Identity
    Alu = mybir.AluOpType

    with tile.TileContext(nc) as tc:
        with (
            tc.tile_pool(name="const", bufs=1) as cpool,
            tc.tile_pool(name="ph", bufs=3, space="PSUM") as php,
            tc.tile_pool(name="acc", bufs=1, space="PSUM") as accp,
            tc.tile_pool(name="rnd", bufs=3) as rnp,
            tc.tile_pool(name="mwork", bufs=3) as vwp,
            tc.tile_pool(name="ework", bufs=5) as ewp,
            tc.tile_pool(name="resid", bufs=1) as rsp,
        ):
            r2 = cpool.tile([4, NR], BF16, tag="r2")
            tw = cpool.tile([4, KL], BF16, tag="tw")
            w1 = cpool.tile([128, NCH, 64], F8, tag="w1")
            w2 = cpool.tile([128, NCH, 64], F8, tag="w2")
            nc.sync.dma_start(r2[:], r2_d[:])
            nc.sync.dma_start(tw[:], tw_d[:])
            nc.gpsimd.dma_start(w1[:], w1_d[:])
            nc.gpsimd.dma_start(w2[:], w2_d[:])

            bias_cos = cpool.tile([128, 1], F32, tag="bcos")
            nc.vector.memset(bias_cos[:], PI / 2)
            bias_magic = cpool.tile([128, 1], F32, tag="bmag")
            nc.vector.memset(bias_magic[:], MAGIC)

            ps = accp.tile([64, KL], F32, tag="ps")

            # supertiles: sins batched over `sc` chunks each
            SUPS = []
            c0 = 0
            while c0 < NCH:
                sc = min(2, NCH - c0)
                SUPS.append((c0, sc))
                c0 += sc
            NSUP = len(SUPS)
            DR = mybir.MatmulPerfMode.DoubleRow
            etiles = {}

            vtiles = {}

            def emit_pre(s):
                c0, sc = SUPS[s]
                # planes: [0,1] = |m| (-> er), [2,3] = m (-> ei)
                vf = vwp.tile([128, 4, KL], F16, tag="vf")
                vtiles[s] = vf
                for h in range(sc):
                    c = c0 + h
                    lhs4 = r2[0:4, c * 128:(c + 1) * 128]
                    ph = php.tile([128, KL], F32, tag="ph")
                    for q in range(2):
                        sl = slice(q * 512, (q + 1) * 512)
                        nc.tensor.matmul(ph[:, sl], lhs4, tw[0:4, sl],
                                         start=True, stop=True)
                    rndS = rnp.tile([128, KL], F32, tag="rnd")
                    if (c * 4) % 9 < 4:
                        nc.scalar.activation(rndS[:], ph[:], Ident,
                                             bias=bias_magic[:], scale=1.0)
                    else:
                        nc.vector.tensor_scalar(rndS[:], ph[:], MAGIC, None,
                                                op0=Alu.add)
                    nc.vector.scalar_tensor_tensor(
                        vf[:, 2 + h, :], rndS[:], MAGIC, ph[:],
                        op0=Alu.subtract, op1=Alu.subtract)
                    nc.vector.tensor_scalar(
                        vf[:, h, :].bitcast(U16),
                        vf[:, 2 + h, :].bitcast(U16),
                        0x7FFF, None, op0=Alu.bitwise_and)

            def emit_sins(s):
                c0, sc = SUPS[s]
                vf = vtiles.pop(s)
                ee = ewp.tile([128, 4, KL], F8, tag="ee")
                etiles[s] = ee
                nc.scalar.activation(ee[:, 2:4, :], vf[:, 2:4, :], Sin,
                                     bias=0.0, scale=2 * PI)
                nc.scalar.activation(ee[:, 0:2, :], vf[:, 0:2, :], Sin,
                                     bias=bias_cos[:], scale=-2 * PI)

            def emit_back(s):
                c0, sc = SUPS[s]
                ee = etiles.pop(s)
                first = s == 0
                last = s == NSUP - 1
                wsl = slice(c0, c0 + 2)
                for j in range(2):
                    sl = slice(j * 512, (j + 1) * 512)
                    nc.tensor.matmul(ps[:, sl], w1[:, wsl, :],
                                     ee[:, 0:2, sl],
                                     start=first, stop=False, perf_mode=DR)
                for j in range(2):
                    sl = slice(j * 512, (j + 1) * 512)
                    nc.tensor.matmul(ps[:, sl], w2[:, wsl, :],
                                     ee[:, 2:4, sl],
                                     start=False, stop=last, perf_mode=DR)

            PF = 2
            for t in range(NSUP + 1 + PF):
                if t < NSUP:
                    emit_pre(t)
                if 1 <= t <= NSUP:
                    emit_sins(t - 1)
                if t >= 1 + PF:
                    emit_back(t - 1 - PF)

            # residual moved to host: just transit ps PSUM->SBUF and DMA
            psS = rsp.tile([64, KL], F32, tag="psS")
            for j in range(2):
                sl = slice(j * 512, (j + 1) * 512)
                nc.vector.tensor_scalar(psS[:, sl], ps[:, sl], 0.0, None,
                                        op0=Alu.add)
            nc.sync.dma_start(pso_d[:], psS[:])

    nc.compile()
    return nc


_NC_CACHE = []


def _get_nc():
    if not _NC_CACHE:
        _NC_CACHE.append(build_kernel())
    return _NC_CACHE[0]


def _host_prep(images_reconstructed, kspace_trajectory, kspace_data,
               kspace_mask, sensitivity_maps):
    img = np.asarray(images_reconstructed)
    traj = np.asarray(kspace_trajectory).astype(np.float32)
    kdata = np.asarray(kspace_data)
    mask = np.asarray(kspace_mask).astype(np.float32)
    smaps = np.asarray(sensitivity_maps)
    bf = ml_dtypes.bfloat16

    x = 0.5 * img[None, ...] * smaps[..., None, None]      # (C,X,Y,Z,S,T)
    xw = x.reshape(C, N, T).transpose(1, 0, 2).reshape(N, CST)

    # mirror pairing: E(-r) = conj(E(r))
    GX, GY = np.meshgrid(np.arange(X) - 48, np.arange(Y) - 48, indexing="ij")
    gxf, gyf = GX.ravel(), GY.ravel()
    n_arr = np.arange(N)
    has_m = (gxf >= -47) & (gyf >= -47)
    mirror_n = np.where(has_m, (48 - gxf) * 96 + (48 - gyf), -1)
    is_rep = (~has_m) | (n_arr <= mirror_n)
    idx = n_arr[is_rep]
    midx = mirror_n[is_rep]
    midx = np.where(midx == idx, -1, midx)
    pad = NR - len(idx)

    xr = xw.real.astype(np.float32)
    xi = xw.imag.astype(np.float32)
    sel = np.maximum(midx, 0)
    on = (midx[:, None] >= 0)
    xr_m = np.where(on, xr[sel], 0.0)
    xi_m = np.where(on, xi[sel], 0.0)
    w1 = np.concatenate([xr[idx] + xr_m, xi[idx] + xi_m], 1)   # [u+; v+]
    w2 = np.concatenate([-(xi[idx] - xi_m), xr[idx] - xr_m], 1)  # [-v-; u-]
    zpad = np.zeros((pad, 64), np.float32)
    f8 = ml_dtypes.float8_e4m3
    w1 = np.ascontiguousarray(np.vstack([w1, zpad]).astype(f8)
                              .reshape(NCH, 128, 64).transpose(1, 0, 2))
    w2 = np.ascontiguousarray(np.vstack([w2, zpad]).astype(f8)
                              .reshape(NCH, 128, 64).transpose(1, 0, 2))

    gxr = np.concatenate([gxf[is_rep], np.zeros(pad)]).astype(np.float32)
    gyr = np.concatenate([gyf[is_rep], np.zeros(pad)]).astype(np.float32)
    r2 = np.stack([gxr, gxr, gyr, gyr]).astype(bf)

    t2 = traj[:2]
    th = t2.astype(bf)
    tl = (t2 - th.astype(np.float32)).astype(bf)
    tw5 = np.stack([th[0], tl[0], th[1], tl[1]])

    mk = mask.reshape(K).astype(np.float32)
    kd = kdata.reshape(C, K, T).transpose(1, 0, 2).reshape(K, CST)
    kdm = kd * mk[:, None]

    in_maps = []
    for i in range(NCORES):
        ksl = slice(i * KL, (i + 1) * KL)
        in_maps.append({
            "w1": w1, "w2": w2, "r2": r2,
            "tw": np.ascontiguousarray(tw5[:, ksl]),
        })
    return in_maps, kdm, mk


def kernel(images_reconstructed, kspace_trajectory, kspace_data,
           kspace_mask, sensitivity_maps, _trace=False):
    nc = _get_nc()
    in_maps, kdm, mk = _host_prep(images_reconstructed, kspace_trajectory,
                                  kspace_data, kspace_mask, sensitivity_maps)
    kw = {"tmpdir": "/tmp/bass_trace"} if _trace else {}
    res = run_bass_kernel_spmd(nc, in_maps, core_ids=list(range(NCORES)),
                               trace=_trace, **kw)
    pso = np.concatenate([res.results[i]["pso"] for i in range(NCORES)],
                         axis=1)                       # (64, K)
    ksp = (pso[:CST] + 1j * pso[CST:]).T.astype(np.complex128)  # (K, CST)
    d = ksp * mk[:, None] - kdm
    ad = np.abs(d)
    l1, l2 = ad.sum(), (ad * ad).sum()
    a = np.abs(kdm)
    a1, a2 = a.sum(), (a * a).sum()
    loss = np.asarray(W1 * (l1 / a1) + W2 * math.sqrt(l2) / math.sqrt(a2),
                      dtype=np.float32)
    if _trace:
        return loss, res
    return loss


# revision 17
# speedup vs baseline: 1.0369x; 1.0369x over previous
"""Trainium2 Bass kernel for nn_KSpaceLoss: exact type-2 NUFFT k-space loss.

loss = 0.1 * (sum|d| / sum|a|) + 0.1 * sqrt(sum d^2 / sum a^2)
  d = (E @ x) * mask - kdata * mask,  a = kdata * mask
  E[k, n] = exp(-2j*pi * traj[:, k] . r[:, n])   (K=8192, N=96*96)

Sharding: K axis split across 8 NeuronCores (1024 samples each).

v2 structure (per core):
 - Mirror pairing: E(-r) = conj(E(r)); host pairs grid points r/-r, so only
   4704 representative points (38 chunks of 128, padded) need phase/trig.
   Paired contribution with u± = xr ± xr', v± = xi ± xi':
     Re += er*u+ + ei*(-v-) ;  Im += er*v+ + ei*u-
 - PE: ph = 4-row split-precision phase matmul (fp32 PSUM)
       rnd = 5-row matmul with MAGIC row last -> fl(phase+MAGIC) exactly
 - VE: mf = (rnd - MAGIC) - ph = -frac(phase)  (f16), uu = |mf| (bitand)
 - ACT: er = sin(pi/2 - 2pi*uu) = cos(2pi*ph); ei = sin(2pi*mf) = -sin(2pi*ph)
 - PE: ps[0:64, k] accumulates er-stream @ [u+;v+] and ei-stream @ [-v-;u-]
 - residual: d = ps*mask - kdata*mask on (64,k); L2 partials via accum;
   |d| via fp32 pairing matmul (dr^2+di^2) + sqrt-accum. a-side sums on host.
"""

import math

import numpy as np
import ml_dtypes

import concourse.bacc as bacc
import concourse.tile as tile
from concourse import mybir
from concourse.bass_utils import run_bass_kernel_spmd

X, Y, Z = 96, 96, 1
C, S, T = 8, 1, 4
K = 8192
N = X * Y * Z
NCORES = 8
KL = K // NCORES          # 1024 k-samples per core
NR = 4864                 # padded representative points (38 chunks)
NCH = NR // 128           # 38
SC = 2                    # chunks per sin supertile
CST = C * S * T           # 32
W1, W2 = 0.1, 0.1

F32 = mybir.dt.float32
F8 = mybir.dt.float8e4
U16 = mybir.dt.uint16
F16 = mybir.dt.float16
BF16 = mybir.dt.bfloat16
PI = math.pi
MAGIC = 12582912.0          # 1.5 * 2^23: fl(x + MAGIC) - MAGIC == round(x)


def build_kernel():
    nc = bacc.Bacc("TRN2", target_bir_lowering=False, debug=False,
                   num_devices=NCORES)

    w1_d = nc.dram_tensor("w1", [128, NCH, 64], F8, kind="ExternalInput").ap()
    w2_d = nc.dram_tensor("w2", [128, NCH, 64], F8, kind="ExternalInput").ap()
    r2_d = nc.dram_tensor("r2", [4, NR], BF16, kind="ExternalInput").ap()
    tw_d = nc.dram_tensor("tw", [4, KL], BF16, kind="ExternalInput").ap()
    pso_d = nc.dram_tensor("pso", [64, KL], F32, kind="ExternalOutput").ap()

    Sin = mybir.ActivationFunctionType.Sin
    Sqrt = mybir.ActivationFunctionType.Sqrt
    Ident = mybir.ActivationFunctionType.Identity
    Alu = mybir.AluOpType

    with tile.TileContext(nc) as tc:
        with (
            tc.tile_pool(name="const", bufs=1) as cpool,
            tc.tile_pool(name="ph", bufs=3, space="PSUM") as php,
            tc.tile_pool(name="acc", bufs=1, space="PSUM") as accp,
            tc.tile_pool(name="rnd", bufs=3) as rnp,
            tc.tile_pool(name="mwork", bufs=3) as vwp,
            tc.tile_pool(name="ework", bufs=5) as ewp,
            tc.tile_pool(name="resid", bufs=1) as rsp,
        ):
            r2 = cpool.tile([4, NR], BF16, tag="r2")
            tw = cpool.tile([4, KL], BF16, tag="tw")
            w1 = cpool.tile([128, NCH, 64], F8, tag="w1")
            w2 = cpool.tile([128, NCH, 64], F8, tag="w2")
            nc.sync.dma_start(r2[:], r2_d[:])
            nc.sync.dma_start(tw[:], tw_d[:])
            nc.gpsimd.dma_start(w1[:], w1_d[:])
            nc.gpsimd.dma_start(w2[:], w2_d[:])

            bias_cos = cpool.tile([128, 1], F32, tag="bcos")
            nc.vector.memset(bias_cos[:], PI / 2)
            bias_magic = cpool.tile([128, 1], F32, tag="bmag")
            nc.vector.memset(bias_magic[:], MAGIC)

            ps = accp.tile([64, KL], F32, tag="ps")

            # supertiles: sins batched over `sc` chunks each
            SUPS = []
            c0 = 0
            while c0 < NCH:
                sc = min(2, NCH - c0)
                SUPS.append((c0, sc))
                c0 += sc
            NSUP = len(SUPS)
            DR = mybir.MatmulPerfMode.DoubleRow
            etiles = {}

            vtiles = {}

            def emit_pre(s):
                c0, sc = SUPS[s]
                # planes: [0,1] = |m| (-> er), [2,3] = m (-> ei)
                vf = vwp.tile([128, 4, KL], F16, tag="vf")
                vtiles[s] = vf
                for h in range(sc):
                    c = c0 + h
                    lhs4 = r2[0:4, c * 128:(c + 1) * 128]
                    ph = php.tile([128, KL], F32, tag="ph")
                    for q in range(2):
                        sl = slice(q * 512, (q + 1) * 512)
                        nc.tensor.matmul(ph[:, sl], lhs4, tw[0:4, sl],
                                         start=True, stop=True)
                    rndS = rnp.tile([128, KL], F32, tag="rnd")
                    if c % 5 in (0, 2):
                        nc.scalar.activation(rndS[:], ph[:], Ident,
                                             bias=bias_magic[:], scale=1.0)
                    else:
                        nc.vector.tensor_scalar(rndS[:], ph[:], MAGIC, None,
                                                op0=Alu.add)
                    nc.vector.scalar_tensor_tensor(
                        vf[:, 2 + h, :], rndS[:], MAGIC, ph[:],
                        op0=Alu.subtract, op1=Alu.subtract)
                    nc.vector.tensor_scalar(
                        vf[:, h, :].bitcast(U16),
                        vf[:, 2 + h, :].bitcast(U16),
                        0x7FFF, None, op0=Alu.bitwise_and)

            def emit_sins(s):
                c0, sc = SUPS[s]
                vf = vtiles.pop(s)
                ee = ewp.tile([128, 4, KL], F8, tag="ee")
                etiles[s] = ee
                nc.scalar.activation(ee[:, 2:4, :], vf[:, 2:4, :], Sin,
                                     bias=0.0, scale=2 * PI)
                nc.scalar.activation(ee[:, 0:2, :], vf[:, 0:2, :], Sin,
                                     bias=bias_cos[:], scale=-2 * PI)

            def emit_back(s):
                c0, sc = SUPS[s]
                ee = etiles.pop(s)
                first = s == 0
                last = s == NSUP - 1
                wsl = slice(c0, c0 + 2)
                for j in range(2):
                    sl = slice(j * 512, (j + 1) * 512)
                    nc.tensor.matmul(ps[:, sl], w1[:, wsl, :],
                                     ee[:, 0:2, sl],
                                     start=first, stop=False, perf_mode=DR)
                for j in range(2):
                    sl = slice(j * 512, (j + 1) * 512)
                    nc.tensor.matmul(ps[:, sl], w2[:, wsl, :],
                                     ee[:, 2:4, sl],
                                     start=False, stop=last, perf_mode=DR)

            PF = 2
            for t in range(NSUP + 1 + PF):
                if t < NSUP:
                    emit_pre(t)
                if 1 <= t <= NSUP:
                    emit_sins(t - 1)
                if t >= 1 + PF:
                    emit_back(t - 1 - PF)

            # residual moved to host: just transit ps PSUM->SBUF and DMA
            psS = rsp.tile([64, KL], F32, tag="psS")
            for j in range(2):
                sl = slice(j * 512, (j + 1) * 512)
                nc.vector.tensor_scalar(psS[:, sl], ps[:, sl], 0.0, None,
                                        op0=Alu.add)
            nc.sync.dma_start(pso_d[:], psS[:])

    nc.compile()
    return nc


_NC_CACHE = []


def _get_nc():
    if not _NC_CACHE:
        _NC_CACHE.append(build_kernel())
    return _NC_CACHE[0]


def _host_prep(images_reconstructed, kspace_trajectory, kspace_data,
               kspace_mask, sensitivity_maps):
    img = np.asarray(images_reconstructed)
    traj = np.asarray(kspace_trajectory).astype(np.float32)
    kdata = np.asarray(kspace_data)
    mask = np.asarray(kspace_mask).astype(np.float32)
    smaps = np.asarray(sensitivity_maps)
    bf = ml_dtypes.bfloat16

    x = 0.5 * img[None, ...] * smaps[..., None, None]      # (C,X,Y,Z,S,T)
    xw = x.reshape(C, N, T).transpose(1, 0, 2).reshape(N, CST)

    # mirror pairing: E(-r) = conj(E(r))
    GX, GY = np.meshgrid(np.arange(X) - 48, np.arange(Y) - 48, indexing="ij")
    gxf, gyf = GX.ravel(), GY.ravel()
    n_arr = np.arange(N)
    has_m = (gxf >= -47) & (gyf >= -47)
    mirror_n = np.where(has_m, (48 - gxf) * 96 + (48 - gyf), -1)
    is_rep = (~has_m) | (n_arr <= mirror_n)
    idx = n_arr[is_rep]
    midx = mirror_n[is_rep]
    midx = np.where(midx == idx, -1, midx)
    pad = NR - len(idx)

    xr = xw.real.astype(np.float32)
    xi = xw.imag.astype(np.float32)
    sel = np.maximum(midx, 0)
    on = (midx[:, None] >= 0)
    xr_m = np.where(on, xr[sel], 0.0)
    xi_m = np.where(on, xi[sel], 0.0)
    w1 = np.concatenate([xr[idx] + xr_m, xi[idx] + xi_m], 1)   # [u+; v+]
    w2 = np.concatenate([-(xi[idx] - xi_m), xr[idx] - xr_m], 1)  # [-v-; u-]
    zpad = np.zeros((pad, 64), np.float32)
    f8 = ml_dtypes.float8_e4m3
    w1 = np.ascontiguousarray(np.vstack([w1, zpad]).astype(f8)
                              .reshape(NCH, 128, 64).transpose(1, 0, 2))
    w2 = np.ascontiguousarray(np.vstack([w2, zpad]).astype(f8)
                              .reshape(NCH, 128, 64).transpose(1, 0, 2))

    gxr = np.concatenate([gxf[is_rep], np.zeros(pad)]).astype(np.float32)
    gyr = np.concatenate([gyf[is_rep], np.zeros(pad)]).astype(np.float32)
    r2 = np.stack([gxr, gxr, gyr, gyr]).astype(bf)

    t2 = traj[:2]
    th = t2.astype(bf)
    tl = (t2 - th.astype(np.float32)).astype(bf)
    tw5 = np.stack([th[0], tl[0], th[1], tl[1]])

    mk = mask.reshape(K).astype(np.float32)
    kd = kdata.reshape(C, K, T).transpose(1, 0, 2).reshape(K, CST)
    kdm = kd * mk[:, None]

    in_maps = []
    for i in range(NCORES):
        ksl = slice(i * KL, (i + 1) * KL)
        in_maps.append({
            "w1": w1, "w2": w2, "r2": r2,
            "tw": np.ascontiguousarray(tw5[:, ksl]),
        })
    return in_maps, kdm, mk


def kernel(images_reconstructed, kspace_trajectory, kspace_data,
           kspace_mask, sensitivity_maps, _trace=False):
    nc = _get_nc()
    in_maps, kdm, mk = _host_prep(images_reconstructed, kspace_trajectory,
                                  kspace_data, kspace_mask, sensitivity_maps)
    kw = {"tmpdir": "/tmp/bass_trace"} if _trace else {}
    res = run_bass_kernel_spmd(nc, in_maps, core_ids=list(range(NCORES)),
                               trace=_trace, **kw)
    pso = np.concatenate([res.results[i]["pso"] for i in range(NCORES)],
                         axis=1)                       # (64, K)
    ksp = (pso[:CST] + 1j * pso[CST:]).T.astype(np.complex128)  # (K, CST)
    d = ksp * mk[:, None] - kdm
    ad = np.abs(d)
    l1, l2 = ad.sum(), (ad * ad).sum()
    a = np.abs(kdm)
    a1, a2 = a.sum(), (a * a).sum()
    loss = np.asarray(W1 * (l1 / a1) + W2 * math.sqrt(l2) / math.sqrt(a2),
                      dtype=np.float32)
    if _trace:
        return loss, res
    return loss
